# Initial kernel scaffold
#
"""BlockDropout kernel for TRN2 (Bass/Tile), data-parallel over 8 NeuronCores.

Problem: z [128, 256, 1024] f32, noise [128, 1024] f32, fallback_idx [128] int.
  mask[b, d] = (noise[b, d] < 0.8); if a row of mask is all zero, force
  mask[b, fallback_idx[b]] = 1.  out[b, m, d] = mask[b, d] * z[b, m, d].

Sharding: batch dim split 8 ways (16 batches per core); no communication.

Per-core device kernel:
  - mask generation on DVE from the noise shard ([16, 1024] ops, tiny),
    force-nonzero handled with a host-provided one-hot of fallback_idx
    (row-max reduce -> all-zero flag -> max with scaled one-hot),
  - per batch, the mask row is broadcast across the 128 SBUF partitions with
    a K=1 matmul on the (otherwise idle) PE into PSUM,
  - per batch, one [128, 2048] f32 tile holds all of z[b] (each partition has
    two of the 256 M-rows), loaded with a single 1 MiB DMA, multiplied on DVE
    against the PSUM mask, stored with a single 1 MiB DMA.
Loads are issued from SP (nc.sync) and stores from ACT (nc.scalar) so the two
HWDGE rings don't head-of-line block each other.
"""

import numpy as np

B, M, D = 128, 256, 1024
NCORES = 8
B_LOC = B // NCORES  # 16 batches per core
FREE = 2 * D         # 2048: two M-rows per SBUF partition => z[b] is [128, FREE]
KEEP = 0.8           # 1 - p_drop

_NC_CACHE = {}


def _build_bass():
    import concourse.bass as bass
    import concourse.mybir as mybir
    import concourse.tile as tile

    f32 = mybir.dt.float32
    nc = bass.Bass(
        "TRN2", target_bir_lowering=False, debug=False, num_devices=NCORES
    )
    z_d = nc.dram_tensor("z", [B_LOC, 128, FREE], f32, kind="ExternalInput")
    noise_d = nc.dram_tensor("noise", [B_LOC, D], f32, kind="ExternalInput")
    onehot_d = nc.dram_tensor("onehot", [B_LOC, D], f32, kind="ExternalInput")
    out_d = nc.dram_tensor("out", [B_LOC, 128, FREE], f32, kind="ExternalOutput")

    with tile.TileContext(nc) as tc:
        with (
            tc.tile_pool(name="const", bufs=1) as cpool,
            tc.tile_pool(name="zp", bufs=4) as zpool,
            tc.tile_pool(name="op", bufs=4) as opool,
            tc.tile_pool(name="mp", bufs=2, space=bass.MemorySpace.PSUM) as mpool,
        ):
            noise_t = cpool.tile([B_LOC, D], f32)
            nc.sync.dma_start(noise_t[:], noise_d.ap())
            oh_t = cpool.tile([B_LOC, D], f32)
            nc.sync.dma_start(oh_t[:], onehot_d.ap())
            ones_t = cpool.tile([1, 128], f32)
            nc.vector.memset(ones_t[:], 1.0)

            # mask = (noise < 0.8) as 1.0/0.0
            mask_t = cpool.tile([B_LOC, D], f32)
            nc.vector.tensor_scalar(
                mask_t[:], noise_t[:], KEEP, None, mybir.AluOpType.is_lt
            )
            # all-zero rows: rowmax == 0
            rowmax_t = cpool.tile([B_LOC, 1], f32)
            nc.vector.tensor_reduce(
                rowmax_t[:], mask_t[:], mybir.AxisListType.X, mybir.AluOpType.max
            )
            allzero_t = cpool.tile([B_LOC, 1], f32)
            nc.vector.tensor_scalar(
                allzero_t[:], rowmax_t[:], 0.5, None, mybir.AluOpType.is_lt
            )
            # mask = max(mask, onehot * allzero[b])
            adj_t = cpool.tile([B_LOC, D], f32)
            nc.vector.tensor_scalar(
                adj_t[:], oh_t[:], allzero_t[:], None, mybir.AluOpType.mult
            )
            maskf_t = cpool.tile([B_LOC, D], f32)
            nc.vector.tensor_tensor(
                maskf_t[:], mask_t[:], adj_t[:], mybir.AluOpType.max
            )

            for b in range(B_LOC):
                zt = zpool.tile([128, FREE], f32)
                nc.sync.dma_start(zt[:], z_d.ap()[b])
                # broadcast mask row b across 128 partitions: ones[1,128].T @ mask[1,512]
                pm = mpool.tile([128, FREE], f32)
                for j in range(4):
                    nc.tensor.matmul(
                        pm[:, j * 512 : (j + 1) * 512],
                        ones_t[0:1, :],
                        maskf_t[b : b + 1, (j % 2) * 512 : ((j % 2) + 1) * 512],
                        start=True,
                        stop=True,
                    )
                ot = opool.tile([128, FREE], f32)
                nc.vector.tensor_mul(ot[:], zt[:], pm[:])
                nc.scalar.dma_start(out_d.ap()[b], ot[:])
    return nc


def get_nc():
    if "nc" not in _NC_CACHE:
        _NC_CACHE["nc"] = _build_bass()
    return _NC_CACHE["nc"]


def kernel(z, noise, fallback_idx):
    from concourse.bass_utils import run_bass_kernel_spmd

    z = np.ascontiguousarray(np.asarray(z, dtype=np.float32))
    noise = np.ascontiguousarray(np.asarray(noise, dtype=np.float32))
    fidx = np.asarray(fallback_idx).astype(np.int64)
    assert z.shape == (B, M, D) and noise.shape == (B, D) and fidx.shape == (B,)

    onehot = (np.arange(D, dtype=np.int64)[None, :] == fidx[:, None]).astype(
        np.float32
    )

    nc = get_nc()
    in_maps = []
    for c in range(NCORES):
        sl = slice(c * B_LOC, (c + 1) * B_LOC)
        in_maps.append(
            {
                "z": z[sl].reshape(B_LOC, 128, FREE),
                "noise": noise[sl],
                "onehot": onehot[sl],
            }
        )
    res = run_bass_kernel_spmd(nc, in_maps, core_ids=list(range(NCORES)))
    outs = [r["out"].reshape(B_LOC, M, D) for r in res.results]
    return np.concatenate(outs, axis=0)


# revision 13
# speedup vs baseline: 1.2391x; 1.2391x over previous
"""BlockDropout kernel for TRN2 (Bass/Tile), data-parallel over 8 NeuronCores.

Problem: z [128, 256, 1024] f32, noise [128, 1024] f32, fallback_idx [128] int.
  mask[b, d] = (noise[b, d] < 0.8); if a row of mask is all zero, force
  mask[b, fallback_idx[b]] = 1.  out[b, m, d] = mask[b, d] * z[b, m, d].

Sharding: batch dim split 8 ways (16 batches per core); no communication.

The force-nonzero fallback is folded into the noise tensor on the host (if a
row of noise is entirely >= 0.8, noise[b, fallback_idx[b]] is set to -1.0,
which forces mask[b, fallback_idx[b]] = 1 on device) — identical to the
reference semantics, and it keeps the device kernel a pure
compare + broadcast + multiply.

Per-core device kernel:
  - mask = (noise < 0.8) computed on DVE straight to bf16 (0/1 exact),
  - mask rows flattened to partition 0 with one SBUF->SBUF DMA,
  - per batch, the mask row is broadcast across the 128 SBUF partitions with
    K=1 bf16 matmuls on the (otherwise idle) PE into PSUM,
  - per batch, one [128, 2048] f32 tile holds all of z[b] (each partition has
    two of the 256 M-rows), loaded with a single 1 MiB DMA, multiplied on DVE
    against the PSUM mask, stored with a single 1 MiB DMA.
Loads are issued from SP (nc.sync) and stores from ACT (nc.scalar) so the two
HWDGE rings don't head-of-line block each other.
"""

import numpy as np

B, M, D = 128, 256, 1024
NCORES = 8
B_LOC = B // NCORES  # 16 batches per core
FREE = 2 * D         # 2048: two M-rows per SBUF partition => z[b] is [128, FREE]
KEEP = 0.8           # 1 - p_drop

_NC_CACHE = {}


def _build_bass(reps=1):
    """Build the per-core module. reps>1 wraps the batch loop in a dynamic
    For_i that redoes the same work (used only for benchmarking)."""
    import contextlib

    import concourse.bass as bass
    import concourse.mybir as mybir
    import concourse.tile as tile
    from concourse import bacc

    f32 = mybir.dt.float32
    bf16 = mybir.dt.bfloat16
    nc = bacc.Bacc(
        "TRN2", target_bir_lowering=False, debug=False, num_devices=NCORES
    )
    z_d = nc.dram_tensor("z", [B_LOC, 128, FREE], f32, kind="ExternalInput")
    noise_d = nc.dram_tensor("noise", [B_LOC, D], f32, kind="ExternalInput")
    out_d = nc.dram_tensor("out", [B_LOC, 128, FREE], f32, kind="ExternalOutput")

    with tile.TileContext(nc) as tc:
        with (
            tc.tile_pool(name="const", bufs=1) as cpool,
            tc.tile_pool(name="zp", bufs=6) as zpool,
            tc.tile_pool(name="op", bufs=6) as opool,
            tc.tile_pool(name="mp", bufs=2, space=bass.MemorySpace.PSUM) as mpool,
        ):
            noise_t = cpool.tile([B_LOC, D], f32)
            nc.sync.dma_start(noise_t[:], noise_d.ap())
            ones_t = cpool.tile([1, 128], bf16)
            nc.vector.memset(ones_t[:], 1.0)

            # mask = (noise < 0.8) as 1.0/0.0, straight to bf16 (exact for 0/1;
            # bf16 runs 4x faster on the PE broadcast matmuls below)
            maskf_t = cpool.tile([B_LOC, D], bf16)
            nc.vector.tensor_scalar(
                maskf_t[:], noise_t[:], KEEP, None, mybir.AluOpType.is_lt
            )
            # flatten all mask rows onto partition 0 so matmul rhs reads are
            # at base partition 0 (HW requires base partition 0/32/64)
            maskrow_t = cpool.tile([1, B_LOC * D], bf16)
            nc.sync.dma_start(maskrow_t[0:1, :], maskf_t[:])

            loop_cm = (
                tc.For_i(0, reps, 1) if reps > 1 else contextlib.nullcontext()
            )
            with loop_cm:
                for b in range(B_LOC):
                    zt = zpool.tile([128, FREE], f32)
                    nc.sync.dma_start(zt[:], z_d.ap()[b])
                    # broadcast mask row b across 128 partitions:
                    # ones[1,128].T @ mask[1,512]
                    pm = mpool.tile([128, FREE], f32)
                    for j in range(4):
                        nc.tensor.matmul(
                            pm[:, j * 512 : (j + 1) * 512],
                            ones_t[0:1, :],
                            maskrow_t[
                                0:1,
                                b * D + (j % 2) * 512 : b * D + (j % 2) * 512 + 512,
                            ],
                            start=True,
                            stop=True,
                        )
                    ot = opool.tile([128, FREE], f32)
                    nc.vector.tensor_mul(ot[:], zt[:], pm[:])
                    nc.scalar.dma_start(out_d.ap()[b], ot[:])
    nc.compile()
    return nc


def get_nc():
    if "nc" not in _NC_CACHE:
        _NC_CACHE["nc"] = _build_bass()
    return _NC_CACHE["nc"]


def _precondition_noise(noise, fidx):
    """Fold the force-nonzero fallback into noise: rows whose mask would be
    all zero get noise[b, fidx[b]] = -1.0 (=> mask 1 at that position)."""
    noise = np.ascontiguousarray(np.asarray(noise, dtype=np.float32)).copy()
    keep = noise < np.float32(KEEP)
    dead = ~keep.any(axis=1)
    if dead.any():
        rows = np.nonzero(dead)[0]
        noise[rows, fidx[rows]] = -1.0
    return noise


def kernel(z, noise, fallback_idx):
    from concourse.bass_utils import run_bass_kernel_spmd

    z = np.ascontiguousarray(np.asarray(z, dtype=np.float32))
    fidx = np.asarray(fallback_idx).astype(np.int64)
    assert z.shape == (B, M, D) and fidx.shape == (B,)
    noise = _precondition_noise(noise, fidx)
    assert noise.shape == (B, D)

    nc = get_nc()
    in_maps = []
    for c in range(NCORES):
        sl = slice(c * B_LOC, (c + 1) * B_LOC)
        in_maps.append(
            {
                "z": z[sl].reshape(B_LOC, 128, FREE),
                "noise": noise[sl],
            }
        )
    res = run_bass_kernel_spmd(nc, in_maps, core_ids=list(range(NCORES)))
    outs = [r["out"].reshape(B_LOC, M, D) for r in res.results]
    return np.concatenate(outs, axis=0)
